# revision 51
# baseline (speedup 1.0000x reference)
"""ConditionGateAttention Trainium2 kernel.

Gated dual-attention block: causal self-attention + cross-attention to a
77-token condition, sigmoid cross-gating, output projection.

  B=2, T=2048, M=77, C=512, H=8 heads, D=64.

Sharding (8 cores): batch x strided-sequence. Core = (b=core//4, j=core%4);
core owns query rows j::4 of its batch, viewed as 4 local blocks of 128
contiguous local rows (= global rows 4*l+j). Under the causal mask every
block bk needs exactly 4*(bk+1) k-tiles on EVERY core, so the program is
SPMD-uniform with zero cross-core padding waste.

Attention output is computed token-major ([q, d] PSUM): the AV matmul uses
the softmax tile as stationary and V as moving (free dim 65 instead of
256), and softmax denominators (ones-column of V) land per-partition so
normalization is a tiny reciprocal + broadcast multiply. The [C, tok]
layout needed by the gate/output matmuls is rebuilt per block with PE
transposes into a block tile ([C-chunk rows, 4 chunks x 128 tok]), which
feeds gates, the z-combine and the output projection entirely per block.

Scheduling: blocks run ascending so K/V projection chunks for block bk+1
are FILLER work interleaved into block bk's exp-bound attention pipeline
(the PE queue is in-order; without filler PE idles while ACT runs exp).
Gates/output for block bk overlap block bk+1's attention. Masks (0/1
multiply on exp output, diagonal groups only) run on DVE in fp16.
Matmul inputs fp16 (full PE rate), fp32 PSUM accumulate.
"""
import numpy as np
import ml_dtypes
from contextlib import ExitStack

import concourse.bass as bass
import concourse.tile as tile
from concourse import bacc, mybir
from concourse import bass_utils

B, T, M, C, H = 2, 2048, 77, 512, 8
D = C // H            # 64
P = 128
KI = C // P           # 4 contraction chunks
PAIRS = H // 2        # 4 head pairs (pair i = heads 2i, 2i+1 = C rows 128i..128i+128)
TQ = T // 4           # local queries per core (512)
NB = TQ // P          # 4 query blocks of 128 local rows
KT = 128              # k-tile size (partition dim of logits)
GROUP = 4             # k-tiles per logits psum group
NEG = -30000.0        # mask bias (exp(-30000+s) == 0)
MP = 128              # condition length M=77 zero-padded to 128 on host
DA = D + 1            # V augmented with a ones-column (denominator col)

f16 = mybir.dt.float16
f32 = mybir.dt.float32
AF = mybir.ActivationFunctionType
ALU = mybir.AluOpType

_cache = {}


def build_program(ext, bias_slots, has_b, stage=4):
    """ext: per-block k-extent in KT tiles (uniform across cores), rounded
    up to GROUP. bias_slots: list of (bk, g) needing a mask tile (uniform;
    data per-core). has_b: dict of which projection biases are nonzero.
    stage: 0=io, 1=projections, 2=+self-attn, 3=+cross, 4=full."""
    key = (tuple(ext), tuple(bias_slots), tuple(sorted(has_b.items())), stage)
    if key in _cache:
        return _cache[key]

    nb = len(bias_slots)
    bias_idx = {ps: n for n, ps in enumerate(bias_slots)}

    nc = bacc.Bacc("TRN2", num_devices=8, debug=False)

    xT_d = nc.dram_tensor("xT", [C, T], f16, kind="ExternalInput").ap()
    xqT_d = nc.dram_tensor("xqT", [C, TQ], f16, kind="ExternalInput").ap()
    cT_d = nc.dram_tensor("cT", [C, MP], f16, kind="ExternalInput").ap()
    w_d = {n: nc.dram_tensor(f"w{n}", [C, C], f16, kind="ExternalInput").ap()
           for n in ["q", "k", "v", "kc", "vc", "g1", "g2", "p"]}
    ident_d = nc.dram_tensor("ident", [P, P], f16, kind="ExternalInput").ap()
    pad_d = nc.dram_tensor("padb", [P, 1], f32, kind="ExternalInput").ap()
    if nb:
        bias_d = nc.dram_tensor("biasm", [nb, P, GROUP * KT], f16,
                                kind="ExternalInput").ap()
    bv_d = {}
    for n in ["q", "k", "kc", "g1", "g2"]:
        if has_b[n]:
            bv_d[n] = nc.dram_tensor(f"b{n}", [P, KI], f32, kind="ExternalInput").ap()
    for n in ["v", "vc", "p"]:
        if has_b[n]:
            bv_d[n] = nc.dram_tensor(f"b{n}", [1, C], f16, kind="ExternalInput").ap()
    out_d = nc.dram_tensor("out", [TQ, C], f16, kind="ExternalOutput").ap()

    def emit(tc, ctx):
        consts = ctx.enter_context(tc.tile_pool(name="consts", bufs=1))
        acts = ctx.enter_context(tc.tile_pool(name="acts", bufs=1))
        work = ctx.enter_context(tc.tile_pool(name="work", bufs=6))
        nrm = ctx.enter_context(tc.tile_pool(name="nrm", bufs=4))
        # PSUM budget (8 banks): ps [P,1024]f32 x2 (4) + y [P,260]f32 x2 (2)
        # + tp [P,128]f16 x2 (2)
        ps_p = ctx.enter_context(tc.tile_pool(name="ps_p", bufs=2, space="PSUM"))
        ps_y = ctx.enter_context(tc.tile_pool(name="ps_y", bufs=2, space="PSUM"))
        ps_t = ctx.enter_context(tc.tile_pool(name="ps_t", bufs=1, space="PSUM"))
        # gates/out-proj accumulators: own bank so the tail chain (drained
        # only when ACT catches up) never blocks the attention lg ring.
        ps_g = ctx.enter_context(tc.tile_pool(name="ps_g", bufs=1, space="PSUM"))

        # ---- load constants/inputs ----
        def chunked(ap):  # [C, n] dram -> [128, 4, n] view
            return ap.rearrange("(o p) n -> p o n", p=P)

        # Two DMA queues: sync(SP) carries the critical early tensors +
        # outputs, gpsimd(Pool) carries the bulk, so issue overhead doesn't
        # serialize the startup. Within each queue, consumption order.
        w_sb = {n: consts.tile([P, KI, C], f16, name=f"w{n}") for n in w_d}
        xqT_sb = consts.tile([P, KI, TQ], f16, name="xqT")
        xT_sb = consts.tile([P, KI, T], f16, name="xT")
        # sync(SP) queue: q inputs first (interleaved per contraction chunk
        # so the first matmul starts after 1/4 of the bytes), then xT in
        # (k-chunk, ki) pieces chunk-0 first, mask tiles before later xT
        # chunks. gpsimd(Pool) queue: the other weights in consumption
        # order. Queues share DMA bandwidth but issue independently.
        for ki in range(KI):
            # first piece on the otherwise-idle DVE queue: parallel issue
            # with SP's preamble at t=0
            q_eng = nc.scalar if ki == 0 else nc.sync
            q_eng.dma_start(xqT_sb[:, ki], chunked(xqT_d)[:, ki])
            nc.gpsimd.dma_start(w_sb["q"][:, ki], chunked(w_d["q"])[:, ki])
        ident = consts.tile([P, P], f16, name="ident")
        nc.sync.dma_start(ident[:], ident_d)
        for n in ["k", "v", "kc", "vc"]:
            nc.gpsimd.dma_start(w_sb[n][:], chunked(w_d[n]))
        cT_sb = consts.tile([P, KI, MP], f16, name="cT")
        pad_sb = consts.tile([P, 1], f32, name="padb")
        if nb:
            bias_sb = consts.tile([P, nb, GROUP * KT], f16, name="biasm")
        for ck in range(KI):
            for ki in range(KI):
                nc.sync.dma_start(xT_sb[:, ki, ck * 512:(ck + 1) * 512],
                                  chunked(xT_d)[:, ki, ck * 512:(ck + 1) * 512])
            if ck == 0:
                nc.sync.dma_start(cT_sb[:], chunked(cT_d))
                nc.sync.dma_start(pad_sb[:], pad_d)
                if nb:
                    nc.sync.dma_start(bias_sb[:],
                                      bias_d.rearrange("n p q -> p n q"))
        for n in ["g1", "g2", "p"]:
            nc.gpsimd.dma_start(w_sb[n][:], chunked(w_d[n]))
        bv_sb = {}
        for n, d in bv_d.items():
            if n in ("v", "vc", "p"):
                bv_sb[n] = consts.tile([P, C], f16, name=f"b{n}")
                nc.sync.dma_start(bv_sb[n][:],
                                  d[0:1, :].unsqueeze(1).to_broadcast((1, P, C)))
            else:
                bv_sb[n] = consts.tile([P, KI], f32, name=f"b{n}")
                nc.sync.dma_start(bv_sb[n][:], d)

        def dump(srcs):
            for m, src in enumerate(srcs):
                osb = work.tile([P, C], f16, tag="osb")
                w = src.shape[-1]
                if w < C:
                    nc.vector.memset(osb[:], 0.0)
                nc.vector.tensor_copy(osb[:, 0:w], src)
                nc.sync.dma_start(out_d[P * m:P * m + P, :], osb[:])

        if stage == 0:
            for m in range(NB):
                osb = work.tile([P, C], f16, tag="osb")
                nc.vector.memset(osb[:], 0.0)
                nc.sync.dma_start(out_d[P * m:P * m + P, :], osb[:])
            return

        # ---- persistent activation tiles ----
        qT_sb = [acts.tile([P, TQ], f16, name=f"qT{i}") for i in range(PAIRS)]
        kT_sb = [acts.tile([P, T], f16, name=f"kT{i}") for i in range(PAIRS)]
        kcT_sb = [acts.tile([P, MP], f16, name=f"kcT{i}") for i in range(PAIRS)]
        v_sb = [acts.tile([P, H * DA], f16, name=f"v{m}") for m in range(T // P)]
        vc_sb = [acts.tile([P, H * DA], f16, name="vc")]
        pct_sb = [acts.tile([P, 2 * TQ], f16, name=f"pct{i}") for i in range(PAIRS)]
        # per-block transposed activations: [C-chunk rows, chunk-major tokens]
        yTB_sb = [acts.tile([P, C], f16, name=f"yTB{bk}") for bk in range(NB)]
        ycTB_sb = [acts.tile([P, C], f16, name=f"ycTB{bk}") for bk in range(NB)]

        # ---- projections ----
        def proj_T(wname, rhs_sb, out_tiles, tt, fw):
            # out[Cout, tt:tt+fw] = W.T @ actT[:, tt:tt+fw]
            for i in range(PAIRS):
                ps = ps_p.tile([P, 1024], f32, tag="ps")
                for ki in range(KI):
                    nc.tensor.matmul(ps[:, 0:fw],
                                     w_sb[wname][:, ki, P * i:P * i + P],
                                     rhs_sb[:, ki, tt:tt + fw],
                                     start=(ki == 0), stop=(ki == KI - 1))
                if has_b[wname]:
                    nc.scalar.activation(out_tiles[i][:, tt:tt + fw], ps[:, 0:fw],
                                         AF.Identity, bias=bv_sb[wname][:, i:i + 1])
                else:
                    nc.vector.tensor_copy(out_tiles[i][:, tt:tt + fw], ps[:, 0:fw])

        # V in natural layout, ones-augmented per head: [tok, H*(D+1)]
        def vproj_tile(wname, src_sb, m, out_tile, pr=P, ones_r=P):
            ps = ps_p.tile([P, 1024], f32, tag="ps")
            if pr < P:
                nc.gpsimd.memset(out_tile[:], 0.0)
            for ki in range(KI):
                nc.tensor.matmul(ps[0:pr, 0:C],
                                 src_sb[:, ki, m * P:m * P + pr],
                                 w_sb[wname][:, ki, :],
                                 start=(ki == 0), stop=(ki == KI - 1))
            dst = out_tile.rearrange("p (h e) -> p h e", e=DA)
            nc.vector.tensor_copy(dst[0:pr, :, 0:D],
                                  ps[0:pr, 0:C].rearrange("p (h e) -> p h e", e=D))
            if has_b[wname]:
                nc.gpsimd.tensor_tensor(
                    dst[0:pr, :, 0:D], dst[0:pr, :, 0:D],
                    bv_sb[wname][0:pr, :].rearrange("p (h e) -> p h e", e=D),
                    ALU.add)
            if ones_r < pr:
                nc.gpsimd.memset(dst[:, :, D:DA], 0.0)
            nc.gpsimd.memset(dst[0:ones_r, :, D:DA], 1.0)

        # filler generator: one item = one pair/tile of the K or V
        # projection for a later k-chunk, emitted inside the attention
        # pipeline to keep PE busy while ACT works through the exps.
        def k_pair_item(ck, i):
            def go():
                ps = ps_p.tile([P, 1024], f32, tag="ps")
                for ki in range(KI):
                    nc.tensor.matmul(ps[:, 0:512],
                                     w_sb["k"][:, ki, P * i:P * i + P],
                                     xT_sb[:, ki, ck * 512:(ck + 1) * 512],
                                     start=(ki == 0), stop=(ki == KI - 1))
                if has_b["k"]:
                    nc.scalar.activation(kT_sb[i][:, ck * 512:(ck + 1) * 512],
                                         ps[:, 0:512], AF.Identity,
                                         bias=bv_sb["k"][:, i:i + 1])
                else:
                    nc.vector.tensor_copy(kT_sb[i][:, ck * 512:(ck + 1) * 512],
                                          ps[:, 0:512])
            return go

        def v_tile_item(m):
            return lambda: vproj_tile("v", xT_sb, m, v_sb[m])

        def chunk_fillers(ck):
            items = []
            for i in range(PAIRS):
                items.append(k_pair_item(ck, i))
            for m in range(ck * GROUP, (ck + 1) * GROUP):
                items.append(v_tile_item(m))
            return items

        # cross-attention logits+exp for all local queries, per pair
        # (both heads merged into one [128,1024] exp). c zero-padded to
        # MP=128 on host: padded K_c/V_c columns are zero, junk logit rows
        # see exp(0)=1 but multiply against zero V_c rows + zero ones-col.
        def cross_logits(i):
            ps = ps_p.tile([P, 1024], f32, tag="ps")
            for hb in range(2):
                b0 = hb * D
                nc.tensor.matmul(ps[:, hb * TQ:(hb + 1) * TQ],
                                 kcT_sb[i][b0:b0 + D, :],
                                 qT_sb[i][b0:b0 + D, :],
                                 start=True, stop=True)
            nc.scalar.activation(pct_sb[i][:], ps[:], AF.Exp, bias=pad_sb[:, 0:1])

        # ---- attention for one query block ----
        def normalize(ps_a, ps_b, dst_sb):
            # bank A holds even heads (slots i=pair), bank B odd heads;
            # col h*DA+64 of each slot is the softmax denominator.
            a3 = ps_a[:, 0:4 * DA].rearrange("p (h e) -> p h e", e=DA)
            b3 = ps_b[:, 0:4 * DA].rearrange("p (h e) -> p h e", e=DA)
            rec = nrm.tile([P, H], f32, tag="rec")
            nc.vector.reciprocal(rec[:, 0:4], a3[:, :, D:DA])
            nc.vector.reciprocal(rec[:, 4:8], b3[:, :, D:DA])
            dst4 = dst_sb.rearrange("p (i two d) -> p i two d", two=2, d=D)
            for idx, src3 in ((0, a3), (1, b3)):
                nc.vector.tensor_tensor(
                    dst4[:, :, idx, :],
                    src3[:, :, 0:D],
                    rec[:, idx * 4:(idx + 1) * 4].unsqueeze(2).to_broadcast(
                        (P, 4, D)),
                    ALU.mult)

        def transposes(src_sb, dst_tile):
            # two halves of one single-bank tile: transpose c4+1 overlaps
            # the DVE drain of c4 (single-shot groups may share a bank)
            tps = ps_t.tile([P, 2 * P], f16, tag="tp")
            for c4 in range(KI):
                h = (c4 % 2) * P
                nc.tensor.transpose(tps[:, h:h + P],
                                    src_sb[:, c4 * P:(c4 + 1) * P], ident[:])
                nc.vector.tensor_copy(dst_tile[:, c4 * P:(c4 + 1) * P],
                                      tps[:, h:h + P])

        def attention_block(bk, do_cross, fillers):
            q0 = bk * P
            ngrp = ext[bk] // GROUP

            def yslice(t_a, t_b, h):
                t = (t_a if h % 2 == 0 else t_b)
                t3 = t[:, 0:4 * DA].rearrange("p (h e) -> p h e", e=DA)
                return t3[:, h // 2, :]

            # cross branch first: independent of self-attention, so its
            # normalize/transposes overlap this block's own pipeline and
            # drop off the end-of-block tail chain.
            ycsb = None
            if do_cross:
                yclo = ps_y.tile([P, 512], f32, tag="y")
                ychi = ps_y.tile([P, 512], f32, tag="y")
                for i in range(PAIRS):
                    for hb in range(2):
                        h = 2 * i + hb
                        nc.tensor.matmul(
                            yslice(yclo, ychi, h),
                            pct_sb[i][:, hb * TQ + q0: hb * TQ + q0 + P],
                            vc_sb[0][:, h * DA:(h + 1) * DA],
                            start=True, stop=True)
                ycsb = work.tile([P, C], f16, tag="ysb")
                normalize(yclo, ychi, ycsb)
                fillers = [lambda: transposes(ycsb, ycTB_sb[bk])] + fillers

            # head-parity split: even heads accumulate in bank A, odd in
            # bank B. Pairs run sequentially, so each bank sees one open
            # accumulation group at a time (zero-region rule; full-bank
            # tiles so groups never share a zero region).
            ylo = ps_y.tile([P, 512], f32, tag="y")
            yhi = ps_y.tile([P, 512], f32, tag="y")

            def qk_group(i, g):
                # logits for k-tiles [4g, 4g+4), both heads of pair i;
                # one exp over [128, 1024].
                lg = ps_p.tile([P, 1024], f32, tag="ps")
                for hb in range(2):
                    b0 = hb * D
                    for s4 in range(GROUP):
                        s = g * GROUP + s4
                        nc.tensor.matmul(
                            lg[:, hb * 512 + s4 * KT: hb * 512 + (s4 + 1) * KT],
                            kT_sb[i][b0:b0 + D, s * KT:(s + 1) * KT],
                            qT_sb[i][b0:b0 + D, q0:q0 + P],
                            start=True, stop=True)
                pt = work.tile([P, 1024], f16, tag="pt")
                nc.scalar.activation(pt[:], lg[:], AF.Exp)
                if (bk, g) in bias_idx:
                    bm = bias_sb[:, bias_idx[(bk, g)], :]
                    nc.vector.tensor_tensor(
                        pt[:].rearrange("p (b q) -> p b q", b=2),
                        pt[:].rearrange("p (b q) -> p b q", b=2),
                        bm.unsqueeze(1).to_broadcast((P, 2, 512)),
                        ALU.mult)
                return pt

            def av_group(i, g, pt):
                for hb in range(2):
                    h = 2 * i + hb
                    for s4 in range(GROUP):
                        s = g * GROUP + s4
                        nc.tensor.matmul(
                            yslice(ylo, yhi, h),
                            pt[:, hb * 512 + s4 * KT: hb * 512 + (s4 + 1) * KT],
                            v_sb[s][:, h * DA:(h + 1) * DA],
                            start=(s == 0), stop=(s == ext[bk] - 1))

            # lag-2 software pipeline over the flattened (pair, group)
            # stream with filler projection work keeping PE fed while ACT
            # works through the exp backlog.
            units = [(i, g) for i in range(PAIRS) for g in range(ngrp)]
            nfill = len(fillers)
            pts = {}
            fi = 0
            half = len(units) // 2 if bk == NB - 1 else 0
            for u, (i, g) in enumerate(units):
                pts[u] = (i, g, qk_group(i, g))
                # spread fillers across units; for the last block back-load
                # them into the second half where the ACT exp backlog peaks
                tgt = max(0, (u + 1 - half)) * nfill // (len(units) - half)
                while fi < tgt:
                    fillers[fi]()
                    fi += 1
                if u >= 4:
                    iu, gu, pt = pts.pop(u - 4)
                    av_group(iu, gu, pt)
            while fi < nfill:
                fillers[fi]()
                fi += 1
            for u in sorted(pts):
                iu, gu, pt = pts.pop(u)
                av_group(iu, gu, pt)

            ysb = work.tile([P, C], f16, tag="ysb")
            normalize(ylo, yhi, ysb)
            return ysb, ycsb

        # ---- per-block tail: transposes, gates, combine, out projection.
        # Returned as closures so they ride as filler inside the NEXT
        # block's attention pipeline (the PE queue is in-order; emitting
        # them inline would stall PE on the DVE/ACT chain).
        # Gates use tanh (same ACT table set as exp — no table reload):
        # sigmoid(x) = (tanh(x/2)+1)/2, with the 1/2 folded into Wp on host.
        def make_tail(bk, ysb, gb, zb, do_gates):
            def f_transpose():
                transposes(ysb, yTB_sb[bk])

            def f_gate(wname, src, combine):
                def go():
                    gps = ps_g.tile([P, 512], f32, tag="gps")
                    for o in range(PAIRS):
                        for i in range(PAIRS):
                            nc.tensor.matmul(gps[:, o * P:(o + 1) * P],
                                             w_sb[wname][:, i, P * o:P * o + P],
                                             src[bk][:, i * P:(i + 1) * P],
                                             start=(i == 0),
                                             stop=(i == PAIRS - 1))
                    g = work.tile([P, C], f16, tag="gb")
                    if has_b[wname]:
                        for o in range(PAIRS):
                            nc.scalar.activation(
                                g[:, o * P:(o + 1) * P],
                                gps[:, o * P:(o + 1) * P], AF.Tanh,
                                bias=bv_sb[wname][:, o:o + 1], scale=0.5)
                    else:
                        nc.scalar.activation(g[:], gps[:, 0:C], AF.Tanh,
                                             scale=0.5)
                    if wname == "g2":
                        # g2's +1 on the idle GPSIMD engine
                        nc.gpsimd.tensor_scalar_add(g[:], g[:], 1.0)
                    else:
                        # g2 (x) yTB on GPSIMD, parallel to g1's PE/ACT path
                        # (yTB is ready: f_transpose ran before this closure)
                        t1 = work.tile([P, C], f16, tag="zt")
                        nc.gpsimd.tensor_tensor(t1[:], gb["g2"][:],
                                                yTB_sb[bk][:], ALU.mult)
                        gb["t1"] = t1
                        nc.vector.tensor_scalar_add(g[:], g[:], 1.0)
                    gb[wname] = g
                    if combine:
                        nc.vector.tensor_tensor(zb[:], gb["g1"][:],
                                                ycTB_sb[bk][:], ALU.mult)
                        nc.vector.tensor_tensor(zb[:], zb[:], gb["t1"][:],
                                                ALU.add)
                return go

            def f_out():
                # two column-halves so the first half's copy+DMA overlaps
                # the second half's matmuls; for the final block the second
                # half drains via the idle ACT engine+queue, shortening the
                # end-of-kernel chain (mid-kernel ACT is exp-saturated).
                last = bk == NB - 1
                osb = work.tile([P, C], f16, tag="osb")
                for half in range(2):
                    c0 = half * (C // 2)
                    alt = last and half == 1
                    ps = ps_g.tile([P, 512], f32, tag="gps")
                    for o in range(PAIRS):
                        nc.tensor.matmul(
                            ps[:, 0:C // 2],
                            zb[:, P * o:P * o + P],
                            w_sb["p"][:, o, c0:c0 + C // 2],
                            start=(o == 0), stop=(o == PAIRS - 1))
                    if has_b["p"]:
                        nc.vector.tensor_tensor(
                            osb[:, c0:c0 + C // 2], ps[:, 0:C // 2],
                            bv_sb["p"][:, c0:c0 + C // 2], ALU.add)
                    elif alt:
                        nc.scalar.activation(osb[:, c0:c0 + C // 2],
                                             ps[:, 0:C // 2], AF.Identity)
                    else:
                        nc.vector.tensor_copy(osb[:, c0:c0 + C // 2],
                                              ps[:, 0:C // 2])
                    q_eng = nc.scalar if alt else nc.sync
                    q_eng.dma_start(out_d[P * bk:P * bk + P, c0:c0 + C // 2],
                                    osb[:, c0:c0 + C // 2])

            if not do_gates:
                return [f_transpose]
            return [f_transpose, f_gate("g1", yTB_sb, True), f_out]

        # ---- top-level schedule ----
        proj_T("q", xqT_sb, qT_sb, 0, TQ)
        do_cross = stage in (3, 4)
        need_chunk = [-(-ext[bk] // GROUP) for bk in range(NB)]
        emitted = 0
        while emitted < need_chunk[0]:
            for it in chunk_fillers(emitted):
                it()
            emitted += 1
        if do_cross or stage == 1:
            proj_T("kc", cT_sb, kcT_sb, 0, MP)
            vproj_tile("vc", cT_sb, 0, vc_sb[0], ones_r=M)
        if do_cross:
            for i in range(PAIRS):
                cross_logits(i)
        if stage == 1:
            dump([qT_sb[0][:, 0:C], kT_sb[0][:, 0:C],
                  v_sb[0][:, 0:C], vc_sb[0][:, 0:C]])
            return
        do_gates = stage == 4

        def emit_g2(bk):
            # g2 = tanh(y_c @ Wg2 / 2): depends only on ycTB (transposed
            # early in the block), so it runs inline at the block's end,
            # keeping PE busy while ACT drains the block's last exps.
            gb = {}
            zb = work.tile([P, C], f16, tag="zb")
            gps = ps_g.tile([P, 512], f32, tag="gps")
            for o in range(PAIRS):
                for i in range(PAIRS):
                    nc.tensor.matmul(gps[:, o * P:(o + 1) * P],
                                     w_sb["g2"][:, i, P * o:P * o + P],
                                     ycTB_sb[bk][:, i * P:(i + 1) * P],
                                     start=(i == 0), stop=(i == PAIRS - 1))
            g = work.tile([P, C], f16, tag="gb")
            if has_b["g2"]:
                for o in range(PAIRS):
                    nc.scalar.activation(g[:, o * P:(o + 1) * P],
                                         gps[:, o * P:(o + 1) * P], AF.Tanh,
                                         bias=bv_sb["g2"][:, o:o + 1], scale=0.5)
            else:
                nc.scalar.activation(g[:], gps[:, 0:C], AF.Tanh, scale=0.5)
            nc.gpsimd.tensor_scalar_add(g[:], g[:], 1.0)
            gb["g2"] = g
            return gb, zb

        tail = []
        for bk in range(NB):
            fillers = list(tail[:3])  # prev block: transposes+gates early
            if bk + 1 < NB:
                for ck in range(emitted, need_chunk[bk + 1]):
                    fillers.extend(chunk_fillers(ck))
                emitted = max(emitted, need_chunk[bk + 1])
            fillers.extend(tail[3:])  # prev block's out-proj lands late
            ysb, ycsb = attention_block(bk, do_cross, fillers)
            gb, zb = (emit_g2(bk) if do_gates else ({}, None))
            tail = make_tail(bk, ysb, gb, zb, do_gates)
        for it in tail:
            it()
        if stage == 2:
            dump([t[:] for t in yTB_sb])
            return
        if stage == 3:
            dump([t[:] for t in ycTB_sb])
            return

    with tile.TileContext(nc) as tc, ExitStack() as ctx:
        emit(tc, ctx)
    nc.compile()
    _cache[key] = nc
    return nc


def prepare(inputs, stage=4):
    """Host-side prep: analyze mask, build program + per-core input maps."""
    x = np.asarray(inputs["x"], np.float32)
    c = np.asarray(inputs["c"], np.float32)
    attn_mask = np.asarray(inputs["attn_mask"])
    padding_mask = np.asarray(inputs["padding_mask"])
    W = {n: np.asarray(inputs["W" + n], np.float32)
         for n in ["q", "k", "v", "kc", "vc", "g1", "g2", "p"]}
    bvec = {n: np.asarray(inputs["b" + n], np.float32)
            for n in ["q", "k", "v", "kc", "vc", "g1", "g2", "p"]}

    scale = 1.0 / np.sqrt(D)
    W = dict(W)
    W["q"] = W["q"] * scale          # fold attention scale into Wq
    bq = bvec["q"] * scale
    # gates computed as tanh: sigmoid(x) = (tanh(x/2)+1)/2 — the kernel
    # applies scale=0.5 inside the activation, so gate biases halve here
    # and the 1/2 of the combine folds into Wp.
    W["p"] = W["p"] * 0.5
    bvec = dict(bvec)
    bvec["g1"] = bvec["g1"] * 0.5
    bvec["g2"] = bvec["g2"] * 0.5

    mask2 = np.asarray(attn_mask).reshape(T, T)  # [q, k]
    # local row l of core j = global row 4*l+j; block bk = local rows
    # [128*bk, 128*bk+128). Extents are maxed over cores (program-uniform).
    rows_of = {j: np.arange(j, T, 4) for j in range(4)}
    ext = []
    last_vis = {}
    for bk in range(NB):
        e = 0
        for j in range(4):
            rr = rows_of[j][bk * P:(bk + 1) * P]
            vis = mask2[rr, :].any(axis=0)
            last = int(np.nonzero(vis)[0].max()) if vis.any() else 0
            last_vis[(bk, j)] = last
            e = max(e, last // KT + 1)
        ext.append(-(-e // GROUP) * GROUP)

    def _slot_needs(bk, s):
        for j in range(4):
            if s > last_vis[(bk, j)] // KT:
                return True
            rr = rows_of[j][bk * P:(bk + 1) * P]
            if not mask2[np.ix_(rr, np.arange(s * KT, (s + 1) * KT))].all():
                return True
        return False

    bias_slots = []
    for bk in range(NB):
        for g in range(ext[bk] // GROUP):
            if any(_slot_needs(bk, g * GROUP + s4) for s4 in range(GROUP)):
                bias_slots.append((bk, g))

    has_b = {n: bool(np.any(bvec[n] != 0)) for n in bvec}
    nc = build_program(ext, bias_slots, has_b, stage=stage)

    w16 = {n: W[n].astype(np.float16) for n in W}
    ident = np.eye(P, dtype=np.float16)
    in_maps = []
    for core in range(8):
        b, j = divmod(core, 4)
        xT = np.ascontiguousarray(x[b].T).astype(np.float16)        # [C, T]
        xqT = np.ascontiguousarray(xT[:, j::4])                     # [C, TQ]
        cT = np.zeros((C, MP), np.float16)
        cT[:, :M] = c[b].T
        pad = np.zeros((P, 1), np.float32)
        pad[:M, 0] = np.where(padding_mask[b] != 0, 0.0, NEG)
        im = {"xT": xT, "xqT": xqT, "cT": cT, "ident": ident, "padb": pad}
        for n in w16:
            im["w" + n] = w16[n]
        if bias_slots:
            bm = np.empty((len(bias_slots), P, GROUP * KT), np.float16)
            for n, (bk, g) in enumerate(bias_slots):
                rr = rows_of[j][bk * P:(bk + 1) * P]
                for e in range(GROUP):
                    s = g * GROUP + e
                    blk = mask2[np.ix_(rr, np.arange(s * KT, (s + 1) * KT))]
                    bm[n, :, e * KT:(e + 1) * KT] = np.where(
                        blk.T, 1.0, 0.0).astype(np.float16)
            im["biasm"] = bm
        for n in ["q", "k", "kc", "g1", "g2"]:
            if has_b[n]:
                v = (bq if n == "q" else bvec[n])
                im["b" + n] = np.ascontiguousarray(
                    v.reshape(KI, P).T).astype(np.float32)
        for n in ["v", "vc", "p"]:
            if has_b[n]:
                im["b" + n] = bvec[n].reshape(1, C).astype(np.float16)
        in_maps.append(im)
    return nc, in_maps


def kernel(**inputs):
    nc, in_maps = prepare(inputs)
    res = bass_utils.run_bass_kernel_spmd(nc, in_maps, core_ids=list(range(8)))
    out = np.empty((B, T, C), np.float32)
    for core in range(8):
        b, j = divmod(core, 4)
        out[b, j::4] = res.results[core]["out"].astype(np.float32)
    return out


# revision 52
# speedup vs baseline: 1.0019x; 1.0019x over previous
"""ConditionGateAttention Trainium2 kernel.

Gated dual-attention block: causal self-attention + cross-attention to a
77-token condition, sigmoid cross-gating, output projection.

  B=2, T=2048, M=77, C=512, H=8 heads, D=64.

Sharding (8 cores): batch x strided-sequence. Core = (b=core//4, j=core%4);
core owns query rows j::4 of its batch, viewed as 4 local blocks of 128
contiguous local rows (= global rows 4*l+j). Under the causal mask every
block bk needs exactly 4*(bk+1) k-tiles on EVERY core, so the program is
SPMD-uniform with zero cross-core padding waste.

Attention output is computed token-major ([q, d] PSUM): the AV matmul uses
the softmax tile as stationary and V as moving (free dim 65 instead of
256), and softmax denominators (ones-column of V) land per-partition so
normalization is a tiny reciprocal + broadcast multiply. The [C, tok]
layout needed by the gate/output matmuls is rebuilt per block with PE
transposes into a block tile ([C-chunk rows, 4 chunks x 128 tok]), which
feeds gates, the z-combine and the output projection entirely per block.

Scheduling: blocks run ascending so K/V projection chunks for block bk+1
are FILLER work interleaved into block bk's exp-bound attention pipeline
(the PE queue is in-order; without filler PE idles while ACT runs exp).
Gates/output for block bk overlap block bk+1's attention. Masks (0/1
multiply on exp output, diagonal groups only) run on DVE in fp16.
Matmul inputs fp16 (full PE rate), fp32 PSUM accumulate.
"""
import numpy as np
import ml_dtypes
from contextlib import ExitStack

import concourse.bass as bass
import concourse.tile as tile
from concourse import bacc, mybir
from concourse import bass_utils

B, T, M, C, H = 2, 2048, 77, 512, 8
D = C // H            # 64
P = 128
KI = C // P           # 4 contraction chunks
PAIRS = H // 2        # 4 head pairs (pair i = heads 2i, 2i+1 = C rows 128i..128i+128)
TQ = T // 4           # local queries per core (512)
NB = TQ // P          # 4 query blocks of 128 local rows
KT = 128              # k-tile size (partition dim of logits)
GROUP = 4             # k-tiles per logits psum group
NEG = -30000.0        # mask bias (exp(-30000+s) == 0)
MP = 128              # condition length M=77 zero-padded to 128 on host
DA = D + 1            # V augmented with a ones-column (denominator col)

f16 = mybir.dt.float16
f32 = mybir.dt.float32
AF = mybir.ActivationFunctionType
ALU = mybir.AluOpType

_cache = {}


def build_program(ext, bias_slots, has_b, stage=4):
    """ext: per-block k-extent in KT tiles (uniform across cores), rounded
    up to GROUP. bias_slots: list of (bk, g) needing a mask tile (uniform;
    data per-core). has_b: dict of which projection biases are nonzero.
    stage: 0=io, 1=projections, 2=+self-attn, 3=+cross, 4=full."""
    key = (tuple(ext), tuple(bias_slots), tuple(sorted(has_b.items())), stage)
    if key in _cache:
        return _cache[key]

    nb = len(bias_slots)
    bias_idx = {ps: n for n, ps in enumerate(bias_slots)}

    nc = bacc.Bacc("TRN2", num_devices=8, debug=False)

    xT_d = nc.dram_tensor("xT", [C, T], f16, kind="ExternalInput").ap()
    xqT_d = nc.dram_tensor("xqT", [C, TQ], f16, kind="ExternalInput").ap()
    cT_d = nc.dram_tensor("cT", [C, MP], f16, kind="ExternalInput").ap()
    w_d = {n: nc.dram_tensor(f"w{n}", [C, C], f16, kind="ExternalInput").ap()
           for n in ["q", "k", "v", "kc", "vc", "g1", "g2", "p"]}
    ident_d = nc.dram_tensor("ident", [P, P], f16, kind="ExternalInput").ap()
    pad_d = nc.dram_tensor("padb", [P, 1], f32, kind="ExternalInput").ap()
    if nb:
        bias_d = nc.dram_tensor("biasm", [nb, P, GROUP * KT], f16,
                                kind="ExternalInput").ap()
    bv_d = {}
    for n in ["q", "k", "kc", "g1", "g2"]:
        if has_b[n]:
            bv_d[n] = nc.dram_tensor(f"b{n}", [P, KI], f32, kind="ExternalInput").ap()
    for n in ["v", "vc", "p"]:
        if has_b[n]:
            bv_d[n] = nc.dram_tensor(f"b{n}", [1, C], f16, kind="ExternalInput").ap()
    out_d = nc.dram_tensor("out", [TQ, C], f16, kind="ExternalOutput").ap()

    def emit(tc, ctx):
        consts = ctx.enter_context(tc.tile_pool(name="consts", bufs=1))
        acts = ctx.enter_context(tc.tile_pool(name="acts", bufs=1))
        work = ctx.enter_context(tc.tile_pool(name="work", bufs=6))
        nrm = ctx.enter_context(tc.tile_pool(name="nrm", bufs=4))
        # PSUM budget (8 banks): ps [P,1024]f32 x2 (4) + y [P,260]f32 x2 (2)
        # + tp [P,128]f16 x2 (2)
        ps_p = ctx.enter_context(tc.tile_pool(name="ps_p", bufs=2, space="PSUM"))
        ps_y = ctx.enter_context(tc.tile_pool(name="ps_y", bufs=2, space="PSUM"))
        ps_t = ctx.enter_context(tc.tile_pool(name="ps_t", bufs=1, space="PSUM"))
        # gates/out-proj accumulators: own bank so the tail chain (drained
        # only when ACT catches up) never blocks the attention lg ring.
        ps_g = ctx.enter_context(tc.tile_pool(name="ps_g", bufs=1, space="PSUM"))

        # ---- load constants/inputs ----
        def chunked(ap):  # [C, n] dram -> [128, 4, n] view
            return ap.rearrange("(o p) n -> p o n", p=P)

        # Two DMA queues: sync(SP) carries the critical early tensors +
        # outputs, gpsimd(Pool) carries the bulk, so issue overhead doesn't
        # serialize the startup. Within each queue, consumption order.
        w_sb = {n: consts.tile([P, KI, C], f16, name=f"w{n}") for n in w_d}
        xqT_sb = consts.tile([P, KI, TQ], f16, name="xqT")
        xT_sb = consts.tile([P, KI, T], f16, name="xT")
        # sync(SP) queue: q inputs first (interleaved per contraction chunk
        # so the first matmul starts after 1/4 of the bytes), then xT in
        # (k-chunk, ki) pieces chunk-0 first, mask tiles before later xT
        # chunks. gpsimd(Pool) queue: the other weights in consumption
        # order. Queues share DMA bandwidth but issue independently.
        for ki in range(KI):
            # first piece on the otherwise-idle DVE queue: parallel issue
            # with SP's preamble at t=0
            q_eng = nc.scalar if ki == 0 else nc.sync
            q_eng.dma_start(xqT_sb[:, ki], chunked(xqT_d)[:, ki])
            nc.gpsimd.dma_start(w_sb["q"][:, ki], chunked(w_d["q"])[:, ki])
        ident = consts.tile([P, P], f16, name="ident")
        nc.sync.dma_start(ident[:], ident_d)
        for n in ["k", "v", "kc", "vc"]:
            nc.gpsimd.dma_start(w_sb[n][:], chunked(w_d[n]))
        cT_sb = consts.tile([P, KI, MP], f16, name="cT")
        pad_sb = consts.tile([P, 1], f32, name="padb")
        if nb:
            bias_sb = consts.tile([P, nb, GROUP * KT], f16, name="biasm")
        for ck in range(KI):
            for ki in range(KI):
                nc.sync.dma_start(xT_sb[:, ki, ck * 512:(ck + 1) * 512],
                                  chunked(xT_d)[:, ki, ck * 512:(ck + 1) * 512])
            if ck == 0:
                nc.sync.dma_start(cT_sb[:], chunked(cT_d))
                nc.sync.dma_start(pad_sb[:], pad_d)
                if nb:
                    nc.sync.dma_start(bias_sb[:],
                                      bias_d.rearrange("n p q -> p n q"))
        for n in ["g1", "g2", "p"]:
            nc.gpsimd.dma_start(w_sb[n][:], chunked(w_d[n]))
        bv_sb = {}
        for n, d in bv_d.items():
            if n in ("v", "vc", "p"):
                bv_sb[n] = consts.tile([P, C], f16, name=f"b{n}")
                nc.sync.dma_start(bv_sb[n][:],
                                  d[0:1, :].unsqueeze(1).to_broadcast((1, P, C)))
            else:
                bv_sb[n] = consts.tile([P, KI], f32, name=f"b{n}")
                nc.sync.dma_start(bv_sb[n][:], d)

        def dump(srcs):
            for m, src in enumerate(srcs):
                osb = work.tile([P, C], f16, tag="osb")
                w = src.shape[-1]
                if w < C:
                    nc.vector.memset(osb[:], 0.0)
                nc.vector.tensor_copy(osb[:, 0:w], src)
                nc.sync.dma_start(out_d[P * m:P * m + P, :], osb[:])

        if stage == 0:
            for m in range(NB):
                osb = work.tile([P, C], f16, tag="osb")
                nc.vector.memset(osb[:], 0.0)
                nc.sync.dma_start(out_d[P * m:P * m + P, :], osb[:])
            return

        # ---- persistent activation tiles ----
        qT_sb = [acts.tile([P, TQ], f16, name=f"qT{i}") for i in range(PAIRS)]
        kT_sb = [acts.tile([P, T], f16, name=f"kT{i}") for i in range(PAIRS)]
        kcT_sb = [acts.tile([P, MP], f16, name=f"kcT{i}") for i in range(PAIRS)]
        v_sb = [acts.tile([P, H * DA], f16, name=f"v{m}") for m in range(T // P)]
        vc_sb = [acts.tile([P, H * DA], f16, name="vc")]
        pct_sb = [acts.tile([P, 2 * TQ], f16, name=f"pct{i}") for i in range(PAIRS)]
        # per-block transposed activations: [C-chunk rows, chunk-major tokens]
        yTB_sb = [acts.tile([P, C], f16, name=f"yTB{bk}") for bk in range(NB)]
        ycTB_sb = [acts.tile([P, C], f16, name=f"ycTB{bk}") for bk in range(NB)]

        # ---- projections ----
        def proj_T(wname, rhs_sb, out_tiles, tt, fw):
            # out[Cout, tt:tt+fw] = W.T @ actT[:, tt:tt+fw]
            for i in range(PAIRS):
                ps = ps_p.tile([P, 1024], f32, tag="ps")
                for ki in range(KI):
                    nc.tensor.matmul(ps[:, 0:fw],
                                     w_sb[wname][:, ki, P * i:P * i + P],
                                     rhs_sb[:, ki, tt:tt + fw],
                                     start=(ki == 0), stop=(ki == KI - 1))
                if has_b[wname]:
                    nc.scalar.activation(out_tiles[i][:, tt:tt + fw], ps[:, 0:fw],
                                         AF.Identity, bias=bv_sb[wname][:, i:i + 1])
                else:
                    nc.vector.tensor_copy(out_tiles[i][:, tt:tt + fw], ps[:, 0:fw])

        # V in natural layout, ones-augmented per head: [tok, H*(D+1)]
        def vproj_tile(wname, src_sb, m, out_tile, pr=P, ones_r=P):
            ps = ps_p.tile([P, 1024], f32, tag="ps")
            if pr < P:
                nc.gpsimd.memset(out_tile[:], 0.0)
            for ki in range(KI):
                nc.tensor.matmul(ps[0:pr, 0:C],
                                 src_sb[:, ki, m * P:m * P + pr],
                                 w_sb[wname][:, ki, :],
                                 start=(ki == 0), stop=(ki == KI - 1))
            dst = out_tile.rearrange("p (h e) -> p h e", e=DA)
            nc.vector.tensor_copy(dst[0:pr, :, 0:D],
                                  ps[0:pr, 0:C].rearrange("p (h e) -> p h e", e=D))
            if has_b[wname]:
                nc.gpsimd.tensor_tensor(
                    dst[0:pr, :, 0:D], dst[0:pr, :, 0:D],
                    bv_sb[wname][0:pr, :].rearrange("p (h e) -> p h e", e=D),
                    ALU.add)
            if ones_r < pr:
                nc.gpsimd.memset(dst[:, :, D:DA], 0.0)
            nc.gpsimd.memset(dst[0:ones_r, :, D:DA], 1.0)

        # filler generator: one item = one pair/tile of the K or V
        # projection for a later k-chunk, emitted inside the attention
        # pipeline to keep PE busy while ACT works through the exps.
        def k_pair_item(ck, i):
            def go():
                ps = ps_p.tile([P, 1024], f32, tag="ps")
                for ki in range(KI):
                    nc.tensor.matmul(ps[:, 0:512],
                                     w_sb["k"][:, ki, P * i:P * i + P],
                                     xT_sb[:, ki, ck * 512:(ck + 1) * 512],
                                     start=(ki == 0), stop=(ki == KI - 1))
                if has_b["k"]:
                    nc.scalar.activation(kT_sb[i][:, ck * 512:(ck + 1) * 512],
                                         ps[:, 0:512], AF.Identity,
                                         bias=bv_sb["k"][:, i:i + 1])
                else:
                    nc.vector.tensor_copy(kT_sb[i][:, ck * 512:(ck + 1) * 512],
                                          ps[:, 0:512])
            return go

        def v_tile_item(m):
            return lambda: vproj_tile("v", xT_sb, m, v_sb[m])

        def chunk_fillers(ck):
            items = []
            for i in range(PAIRS):
                items.append(k_pair_item(ck, i))
            for m in range(ck * GROUP, (ck + 1) * GROUP):
                items.append(v_tile_item(m))
            return items

        # cross-attention logits+exp for all local queries, per pair
        # (both heads merged into one [128,1024] exp). c zero-padded to
        # MP=128 on host: padded K_c/V_c columns are zero, junk logit rows
        # see exp(0)=1 but multiply against zero V_c rows + zero ones-col.
        def cross_logits(i):
            ps = ps_p.tile([P, 1024], f32, tag="ps")
            for hb in range(2):
                b0 = hb * D
                nc.tensor.matmul(ps[:, hb * TQ:(hb + 1) * TQ],
                                 kcT_sb[i][b0:b0 + D, :],
                                 qT_sb[i][b0:b0 + D, :],
                                 start=True, stop=True)
            nc.scalar.activation(pct_sb[i][:], ps[:], AF.Exp, bias=pad_sb[:, 0:1])

        # ---- attention for one query block ----
        def normalize(ps_a, ps_b, dst_sb):
            # bank A holds even heads (slots i=pair), bank B odd heads;
            # col h*DA+64 of each slot is the softmax denominator.
            a3 = ps_a[:, 0:4 * DA].rearrange("p (h e) -> p h e", e=DA)
            b3 = ps_b[:, 0:4 * DA].rearrange("p (h e) -> p h e", e=DA)
            rec = nrm.tile([P, H], f32, tag="rec")
            nc.vector.reciprocal(rec[:, 0:4], a3[:, :, D:DA])
            nc.vector.reciprocal(rec[:, 4:8], b3[:, :, D:DA])
            dst4 = dst_sb.rearrange("p (i two d) -> p i two d", two=2, d=D)
            for idx, src3 in ((0, a3), (1, b3)):
                nc.vector.tensor_tensor(
                    dst4[:, :, idx, :],
                    src3[:, :, 0:D],
                    rec[:, idx * 4:(idx + 1) * 4].unsqueeze(2).to_broadcast(
                        (P, 4, D)),
                    ALU.mult)

        def transposes(src_sb, dst_tile):
            # two halves of one single-bank tile: transpose c4+1 overlaps
            # the DVE drain of c4 (single-shot groups may share a bank)
            tps = ps_t.tile([P, 2 * P], f16, tag="tp")
            for c4 in range(KI):
                h = (c4 % 2) * P
                nc.tensor.transpose(tps[:, h:h + P],
                                    src_sb[:, c4 * P:(c4 + 1) * P], ident[:])
                nc.vector.tensor_copy(dst_tile[:, c4 * P:(c4 + 1) * P],
                                      tps[:, h:h + P])

        def attention_block(bk, do_cross, fillers):
            q0 = bk * P
            ngrp = ext[bk] // GROUP

            def yslice(t_a, t_b, h):
                t = (t_a if h % 2 == 0 else t_b)
                t3 = t[:, 0:4 * DA].rearrange("p (h e) -> p h e", e=DA)
                return t3[:, h // 2, :]

            # cross branch first: independent of self-attention, so its
            # normalize/transposes overlap this block's own pipeline and
            # drop off the end-of-block tail chain.
            ycsb = None
            if do_cross:
                yclo = ps_y.tile([P, 512], f32, tag="y")
                ychi = ps_y.tile([P, 512], f32, tag="y")
                for i in range(PAIRS):
                    for hb in range(2):
                        h = 2 * i + hb
                        nc.tensor.matmul(
                            yslice(yclo, ychi, h),
                            pct_sb[i][:, hb * TQ + q0: hb * TQ + q0 + P],
                            vc_sb[0][:, h * DA:(h + 1) * DA],
                            start=True, stop=True)
                ycsb = work.tile([P, C], f16, tag="ysb")
                normalize(yclo, ychi, ycsb)
                fillers = [lambda: transposes(ycsb, ycTB_sb[bk])] + fillers

            # head-parity split: even heads accumulate in bank A, odd in
            # bank B. Pairs run sequentially, so each bank sees one open
            # accumulation group at a time (zero-region rule; full-bank
            # tiles so groups never share a zero region).
            ylo = ps_y.tile([P, 512], f32, tag="y")
            yhi = ps_y.tile([P, 512], f32, tag="y")

            def qk_group(i, g):
                # logits for k-tiles [4g, 4g+4), both heads of pair i;
                # one exp over [128, 1024].
                lg = ps_p.tile([P, 1024], f32, tag="ps")
                for hb in range(2):
                    b0 = hb * D
                    for s4 in range(GROUP):
                        s = g * GROUP + s4
                        nc.tensor.matmul(
                            lg[:, hb * 512 + s4 * KT: hb * 512 + (s4 + 1) * KT],
                            kT_sb[i][b0:b0 + D, s * KT:(s + 1) * KT],
                            qT_sb[i][b0:b0 + D, q0:q0 + P],
                            start=True, stop=True)
                pt = work.tile([P, 1024], f16, tag="pt")
                nc.scalar.activation(pt[:], lg[:], AF.Exp)
                if (bk, g) in bias_idx:
                    bm = bias_sb[:, bias_idx[(bk, g)], :]
                    nc.vector.tensor_tensor(
                        pt[:].rearrange("p (b q) -> p b q", b=2),
                        pt[:].rearrange("p (b q) -> p b q", b=2),
                        bm.unsqueeze(1).to_broadcast((P, 2, 512)),
                        ALU.mult)
                return pt

            def av_group(i, g, pt):
                for hb in range(2):
                    h = 2 * i + hb
                    for s4 in range(GROUP):
                        s = g * GROUP + s4
                        nc.tensor.matmul(
                            yslice(ylo, yhi, h),
                            pt[:, hb * 512 + s4 * KT: hb * 512 + (s4 + 1) * KT],
                            v_sb[s][:, h * DA:(h + 1) * DA],
                            start=(s == 0), stop=(s == ext[bk] - 1))

            # lag-2 software pipeline over the flattened (pair, group)
            # stream with filler projection work keeping PE fed while ACT
            # works through the exp backlog.
            units = [(i, g) for i in range(PAIRS) for g in range(ngrp)]
            nfill = len(fillers)
            pts = {}
            fi = 0
            half = len(units) // 2 if bk == NB - 1 else 0
            for u, (i, g) in enumerate(units):
                pts[u] = (i, g, qk_group(i, g))
                # spread fillers across units; for the last block back-load
                # them into the second half where the ACT exp backlog peaks
                tgt = max(0, (u + 1 - half)) * nfill // (len(units) - half)
                while fi < tgt:
                    fillers[fi]()
                    fi += 1
                if u >= 4:
                    iu, gu, pt = pts.pop(u - 4)
                    av_group(iu, gu, pt)
            while fi < nfill:
                fillers[fi]()
                fi += 1
            for u in sorted(pts):
                iu, gu, pt = pts.pop(u)
                av_group(iu, gu, pt)

            ysb = work.tile([P, C], f16, tag="ysb")
            normalize(ylo, yhi, ysb)
            return ysb, ycsb

        # ---- per-block tail: transposes, gates, combine, out projection.
        # Returned as closures so they ride as filler inside the NEXT
        # block's attention pipeline (the PE queue is in-order; emitting
        # them inline would stall PE on the DVE/ACT chain).
        # Gates use tanh (same ACT table set as exp — no table reload):
        # sigmoid(x) = (tanh(x/2)+1)/2, with the 1/2 folded into Wp on host.
        def make_tail(bk, ysb, gb, zb, do_gates):
            def f_transpose():
                transposes(ysb, yTB_sb[bk])

            def f_gate(wname, src, combine):
                def go():
                    gps = ps_g.tile([P, 512], f32, tag="gps")
                    for o in range(PAIRS):
                        for i in range(PAIRS):
                            nc.tensor.matmul(gps[:, o * P:(o + 1) * P],
                                             w_sb[wname][:, i, P * o:P * o + P],
                                             src[bk][:, i * P:(i + 1) * P],
                                             start=(i == 0),
                                             stop=(i == PAIRS - 1))
                    g = work.tile([P, C], f16, tag="gb")
                    if has_b[wname]:
                        for o in range(PAIRS):
                            nc.scalar.activation(
                                g[:, o * P:(o + 1) * P],
                                gps[:, o * P:(o + 1) * P], AF.Tanh,
                                bias=bv_sb[wname][:, o:o + 1], scale=0.5)
                    else:
                        nc.scalar.activation(g[:], gps[:, 0:C], AF.Tanh,
                                             scale=0.5)
                    if wname == "g2":
                        # g2's +1 on the idle GPSIMD engine
                        nc.gpsimd.tensor_scalar_add(g[:], g[:], 1.0)
                    else:
                        # g2 (x) yTB on GPSIMD, parallel to g1's PE/ACT path
                        # (yTB is ready: f_transpose ran before this closure)
                        t1 = work.tile([P, C], f16, tag="zt")
                        nc.gpsimd.tensor_tensor(t1[:], gb["g2"][:],
                                                yTB_sb[bk][:], ALU.mult)
                        gb["t1"] = t1
                        nc.vector.tensor_scalar_add(g[:], g[:], 1.0)
                    gb[wname] = g
                    if combine:
                        nc.vector.tensor_tensor(zb[:], gb["g1"][:],
                                                ycTB_sb[bk][:], ALU.mult)
                        nc.vector.tensor_tensor(zb[:], zb[:], gb["t1"][:],
                                                ALU.add)
                return go

            def f_out():
                # two column-halves so the first half's copy+DMA overlaps
                # the second half's matmuls (shortens the exposed tail)
                osb = work.tile([P, C], f16, tag="osb")
                for half in range(2):
                    c0 = half * (C // 2)
                    ps = ps_g.tile([P, 512], f32, tag="gps")
                    for o in range(PAIRS):
                        nc.tensor.matmul(
                            ps[:, 0:C // 2],
                            zb[:, P * o:P * o + P],
                            w_sb["p"][:, o, c0:c0 + C // 2],
                            start=(o == 0), stop=(o == PAIRS - 1))
                    if has_b["p"]:
                        nc.vector.tensor_tensor(
                            osb[:, c0:c0 + C // 2], ps[:, 0:C // 2],
                            bv_sb["p"][:, c0:c0 + C // 2], ALU.add)
                    else:
                        nc.vector.tensor_copy(osb[:, c0:c0 + C // 2],
                                              ps[:, 0:C // 2])
                    nc.sync.dma_start(out_d[P * bk:P * bk + P, c0:c0 + C // 2],
                                      osb[:, c0:c0 + C // 2])

            if not do_gates:
                return [f_transpose]
            return [f_transpose, f_gate("g1", yTB_sb, True), f_out]

        # ---- top-level schedule ----
        proj_T("q", xqT_sb, qT_sb, 0, TQ)
        do_cross = stage in (3, 4)
        need_chunk = [-(-ext[bk] // GROUP) for bk in range(NB)]
        emitted = 0
        while emitted < need_chunk[0]:
            for it in chunk_fillers(emitted):
                it()
            emitted += 1
        if do_cross or stage == 1:
            proj_T("kc", cT_sb, kcT_sb, 0, MP)
            vproj_tile("vc", cT_sb, 0, vc_sb[0], ones_r=M)
        if do_cross:
            for i in range(PAIRS):
                cross_logits(i)
        if stage == 1:
            dump([qT_sb[0][:, 0:C], kT_sb[0][:, 0:C],
                  v_sb[0][:, 0:C], vc_sb[0][:, 0:C]])
            return
        do_gates = stage == 4

        def emit_g2(bk):
            # g2 = tanh(y_c @ Wg2 / 2): depends only on ycTB (transposed
            # early in the block), so it runs inline at the block's end,
            # keeping PE busy while ACT drains the block's last exps.
            gb = {}
            zb = work.tile([P, C], f16, tag="zb")
            gps = ps_g.tile([P, 512], f32, tag="gps")
            for o in range(PAIRS):
                for i in range(PAIRS):
                    nc.tensor.matmul(gps[:, o * P:(o + 1) * P],
                                     w_sb["g2"][:, i, P * o:P * o + P],
                                     ycTB_sb[bk][:, i * P:(i + 1) * P],
                                     start=(i == 0), stop=(i == PAIRS - 1))
            g = work.tile([P, C], f16, tag="gb")
            if has_b["g2"]:
                for o in range(PAIRS):
                    nc.scalar.activation(g[:, o * P:(o + 1) * P],
                                         gps[:, o * P:(o + 1) * P], AF.Tanh,
                                         bias=bv_sb["g2"][:, o:o + 1], scale=0.5)
            else:
                nc.scalar.activation(g[:], gps[:, 0:C], AF.Tanh, scale=0.5)
            nc.gpsimd.tensor_scalar_add(g[:], g[:], 1.0)
            gb["g2"] = g
            return gb, zb

        tail = []
        for bk in range(NB):
            fillers = list(tail[:3])  # prev block: transposes+gates early
            if bk + 1 < NB:
                for ck in range(emitted, need_chunk[bk + 1]):
                    fillers.extend(chunk_fillers(ck))
                emitted = max(emitted, need_chunk[bk + 1])
            fillers.extend(tail[3:])  # prev block's out-proj lands late
            ysb, ycsb = attention_block(bk, do_cross, fillers)
            gb, zb = (emit_g2(bk) if do_gates else ({}, None))
            tail = make_tail(bk, ysb, gb, zb, do_gates)
        for it in tail:
            it()
        if stage == 2:
            dump([t[:] for t in yTB_sb])
            return
        if stage == 3:
            dump([t[:] for t in ycTB_sb])
            return

    with tile.TileContext(nc) as tc, ExitStack() as ctx:
        emit(tc, ctx)
    nc.compile()
    _cache[key] = nc
    return nc


def prepare(inputs, stage=4):
    """Host-side prep: analyze mask, build program + per-core input maps."""
    x = np.asarray(inputs["x"], np.float32)
    c = np.asarray(inputs["c"], np.float32)
    attn_mask = np.asarray(inputs["attn_mask"])
    padding_mask = np.asarray(inputs["padding_mask"])
    W = {n: np.asarray(inputs["W" + n], np.float32)
         for n in ["q", "k", "v", "kc", "vc", "g1", "g2", "p"]}
    bvec = {n: np.asarray(inputs["b" + n], np.float32)
            for n in ["q", "k", "v", "kc", "vc", "g1", "g2", "p"]}

    scale = 1.0 / np.sqrt(D)
    W = dict(W)
    W["q"] = W["q"] * scale          # fold attention scale into Wq
    bq = bvec["q"] * scale
    # gates computed as tanh: sigmoid(x) = (tanh(x/2)+1)/2 — the kernel
    # applies scale=0.5 inside the activation, so gate biases halve here
    # and the 1/2 of the combine folds into Wp.
    W["p"] = W["p"] * 0.5
    bvec = dict(bvec)
    bvec["g1"] = bvec["g1"] * 0.5
    bvec["g2"] = bvec["g2"] * 0.5

    mask2 = np.asarray(attn_mask).reshape(T, T)  # [q, k]
    # local row l of core j = global row 4*l+j; block bk = local rows
    # [128*bk, 128*bk+128). Extents are maxed over cores (program-uniform).
    rows_of = {j: np.arange(j, T, 4) for j in range(4)}
    ext = []
    last_vis = {}
    for bk in range(NB):
        e = 0
        for j in range(4):
            rr = rows_of[j][bk * P:(bk + 1) * P]
            vis = mask2[rr, :].any(axis=0)
            last = int(np.nonzero(vis)[0].max()) if vis.any() else 0
            last_vis[(bk, j)] = last
            e = max(e, last // KT + 1)
        ext.append(-(-e // GROUP) * GROUP)

    def _slot_needs(bk, s):
        for j in range(4):
            if s > last_vis[(bk, j)] // KT:
                return True
            rr = rows_of[j][bk * P:(bk + 1) * P]
            if not mask2[np.ix_(rr, np.arange(s * KT, (s + 1) * KT))].all():
                return True
        return False

    bias_slots = []
    for bk in range(NB):
        for g in range(ext[bk] // GROUP):
            if any(_slot_needs(bk, g * GROUP + s4) for s4 in range(GROUP)):
                bias_slots.append((bk, g))

    has_b = {n: bool(np.any(bvec[n] != 0)) for n in bvec}
    nc = build_program(ext, bias_slots, has_b, stage=stage)

    w16 = {n: W[n].astype(np.float16) for n in W}
    ident = np.eye(P, dtype=np.float16)
    in_maps = []
    for core in range(8):
        b, j = divmod(core, 4)
        xT = np.ascontiguousarray(x[b].T).astype(np.float16)        # [C, T]
        xqT = np.ascontiguousarray(xT[:, j::4])                     # [C, TQ]
        cT = np.zeros((C, MP), np.float16)
        cT[:, :M] = c[b].T
        pad = np.zeros((P, 1), np.float32)
        pad[:M, 0] = np.where(padding_mask[b] != 0, 0.0, NEG)
        im = {"xT": xT, "xqT": xqT, "cT": cT, "ident": ident, "padb": pad}
        for n in w16:
            im["w" + n] = w16[n]
        if bias_slots:
            bm = np.empty((len(bias_slots), P, GROUP * KT), np.float16)
            for n, (bk, g) in enumerate(bias_slots):
                rr = rows_of[j][bk * P:(bk + 1) * P]
                for e in range(GROUP):
                    s = g * GROUP + e
                    blk = mask2[np.ix_(rr, np.arange(s * KT, (s + 1) * KT))]
                    bm[n, :, e * KT:(e + 1) * KT] = np.where(
                        blk.T, 1.0, 0.0).astype(np.float16)
            im["biasm"] = bm
        for n in ["q", "k", "kc", "g1", "g2"]:
            if has_b[n]:
                v = (bq if n == "q" else bvec[n])
                im["b" + n] = np.ascontiguousarray(
                    v.reshape(KI, P).T).astype(np.float32)
        for n in ["v", "vc", "p"]:
            if has_b[n]:
                im["b" + n] = bvec[n].reshape(1, C).astype(np.float16)
        in_maps.append(im)
    return nc, in_maps


def kernel(**inputs):
    nc, in_maps = prepare(inputs)
    res = bass_utils.run_bass_kernel_spmd(nc, in_maps, core_ids=list(range(8)))
    out = np.empty((B, T, C), np.float32)
    for core in range(8):
        b, j = divmod(core, 4)
        out[b, j::4] = res.results[core]["out"].astype(np.float32)
    return out
